# revision 81
# baseline (speedup 1.0000x reference)
"""Causal self-attention (B=2, S=2048, D=1024, H=16) on 8 NeuronCores.

Sharding: data-parallel over batch (2 groups of 4 cores), tensor-parallel
over heads within a group (4 heads / core). Each core computes Q/K/V
projections for its 4 heads, causal attention, and a partial output
projection through its slice of Wo; the 4 partial [2048, 1024] outputs per
batch are summed on the host.

v3 notes (vs v2, 164us -> 115us simulated):
  - PV runs transposed: OT[q, c] = sum_k P[k, q] V'[k, c] with the P tile
    (natural [key, query] layout) as the full-width 128-col stationary and
    V' (64 v-cols + ones-col) streaming only 65 columns per key tile --
    2.3x fewer PE column-cycles for PV, identical numerics. PV^T
    accumulators are [128, 4, 2, 128] f32 = exactly 2 psum banks with
    512B-aligned (q-tile, head) slices; one accumulation group per bank
    (start lazy-zeroes the bank, stop on its last diagonal matmul).
  - Softmax denominators land on the partition (query) axis (the V'
    ones-column), so normalization is reciprocal + per-partition
    tensor_scalar multiplies on DVE; v2's reciprocal-broadcast matmuls
    are gone. Normalized O^T [q, c] transposes back to [c, q] for the
    output projection with 4 PE transpose matmuls (128 cols each)
    through an identity stationary, then one DVE copy PSUM->SBUF.
  - One (p, kt) stream per q-chunk with scores/exp running LAG=4 key
    tiles ahead of PV^T; p0's normalize/transpose emit mid-stream. The
    3rd fast-psum slot (freed by the single-slot accumulator) breaks the
    ST->exp slot round-trip that throttled the PE.
  - q/k land as fp16 (1 cyc/col like f32r but no >=256-free-col rule, so
    diagonal score tiles trim to exactly dq); PT memsets are gone (PV^T
    never reads below-diagonal slices).
  - Startup DMAs split across the SP and ACT HWDGE queues in
    first-needed order (transfer time is charged serially per queue);
    the first projection chain is emitted as quarter chains in
    DMA-arrival order; V' chains emit inside their own chunk's stream,
    keeping PE filler for the ACT-heavy late chunks; the final store
    splits per 512-col half across both DMA queues to shorten the tail.
  - x and [Wq|Wk|Wv] still ship as fp8e4 hi+lo residual pairs (host-
    prepared; W pre-scaled x32) with 3-term DoubleRow projections;
    P = exp(scores) is written straight to bf16.
"""

import numpy as np
import ml_dtypes

import concourse.bass as bass
import concourse.mybir as mybir
import concourse.tile as tile
from concourse.bass_utils import run_bass_kernel_spmd

F32 = mybir.dt.float32
F32R = mybir.dt.float32r
BF16 = mybir.dt.bfloat16
F8 = mybir.dt.float8e4
F16 = mybir.dt.float16
AF = mybir.ActivationFunctionType
DR = mybir.MatmulPerfMode.DoubleRow
DIV = mybir.AluOpType.divide

B, S, D, H = 2, 2048, 1024, 16
DH = D // H              # 64
HL = 4                   # heads per core
CL = HL * DH             # 256 channels per core
G = 4                    # cores per batch group
WSCALE = 32.0            # host pre-scale on Wq/Wk/Wv (fp8 range)
SCALE = (DH ** -0.5) / (WSCALE * WSCALE)   # folded into exp()
NQC = S // 512           # 4 q-chunks of 512
NKT = S // 128           # 16 key tiles of 128


def _split_excess_waits(nc, max_waits=1):
    """walrus in this toolchain rejects instructions carrying more than
    `max_waits` sem waits; split the excess onto preceding same-engine
    NoOps (sound: waits are monotone >= conditions hoisted earlier on
    the same engine)."""
    n_split = 0
    for f in nc.m.functions:
        for bb in f.blocks:
            out = []
            for inst in bb.instructions:
                si = inst.sync_info
                waits = list(si.on_wait) if si is not None and si.on_wait else []
                if len(waits) > max_waits:
                    head, keep = waits[:-max_waits], waits[-max_waits:]
                    for ci, start in enumerate(range(0, len(head), max_waits)):
                        nop = mybir.InstNoOp(
                            name=f"{inst.name}_wsplit{ci}",
                            sync_info=mybir.SyncInfo(
                                on_wait=head[start:start + max_waits],
                                on_update=[],
                            ),
                            engine=inst.engine,
                            bass_nofuse=True,
                        )
                        out.append(nop)
                        n_split += 1
                    si.on_wait = keep
                out.append(inst)
            if n_split:
                bb.instructions.clear()
                for i in out:
                    bb.instructions.append(i)
    return n_split


def _build_nc(split_waits=True):
    nc = bass.Bass()
    xh_d = nc.dram_tensor("xh", [D, S], F8, kind="ExternalInput")
    xl_d = nc.dram_tensor("xl", [D, S], F8, kind="ExternalInput")
    wh_d = nc.dram_tensor("wh", [D, 3 * CL], F8, kind="ExternalInput")
    wl_d = nc.dram_tensor("wl", [D, 3 * CL], F8, kind="ExternalInput")
    wo_d = nc.dram_tensor("wo", [128, 2, D], BF16, kind="ExternalInput")
    mask_d = nc.dram_tensor("mask", [128, 2, 128], BF16, kind="ExternalInput")
    ident_d = nc.dram_tensor("ident", [128, 128], BF16, kind="ExternalInput")
    y_d = nc.dram_tensor("y", [S // 256, 2, 128, 2, 512], BF16,
                         kind="ExternalOutput")

    xh_r = xh_d.rearrange("(a p) s -> p a s", p=128)
    xl_r = xl_d.rearrange("(a p) s -> p a s", p=128)

    with tile.TileContext(nc) as tc:
        with tc.tile_pool(name="persist", bufs=1) as pp:
            # ---- persistent SBUF tensors -------------------------------
            wh_sb = pp.tile([128, 8, 3 * CL], F8)
            wl_sb = pp.tile([128, 8, 3 * CL], F8)
            xh_sb = pp.tile([128, 8, S], F8)
            xl_sb = pp.tile([128, 8, S], F8)
            wo_sb = pp.tile([128, 2, D], BF16)    # pair-major k-tiles
            mask_sb = pp.tile([128, 2, 128], BF16)  # tri m[k,q]=k<=q, x2 heads
            ident_sb = pp.tile([128, 128], BF16)
            qt_sb = [pp.tile([128, S], F16, name=f"qt{p}", tag=f"qt{p}")
                     for p in range(2)]
            kt_sb = [pp.tile([128, S], F16, name=f"kt{p}", tag=f"kt{p}")
                     for p in range(2)]
            # V' per key-tile: 4x[64 v-cols + 1 ones-col], bf16
            vp_sb = pp.tile([128, NKT, 4 * 65], BF16)

            # ones-columns of V': fill everything with 1.0; the V copies
            # below overwrite the 64 data columns of each head block.
            nc.gpsimd.memset(vp_sb[:], 1.0)

            # ---- input DMAs, spread across the SP and ACT HWDGE queues
            # so the first-chunk operands land early; W split QK|V so the
            # first projection chain isn't gated on the V columns ---------
            wh_r = wh_d.rearrange("(a p) m -> p a m", p=128)
            wl_r = wl_d.rearrange("(a p) m -> p a m", p=128)
            # DMA transfer time is charged serially to the issuing engine,
            # so the per-queue ORDER sets when each operand lands. First
            # projection half-chain needs wh[Q]+xh[0:256]; K cols gate the
            # first scores; V cols gate the first V' chain.
            nc.sync.dma_start(wh_sb[:, :, 0:256], wh_r[:, :, 0:256])
            nc.scalar.dma_start(wl_sb[:, :, 0:256], wl_r[:, :, 0:256])
            nc.sync.dma_start(xh_sb[:, :, 0:256], xh_r[:, :, 0:256])
            nc.scalar.dma_start(xl_sb[:, :, 0:256], xl_r[:, :, 0:256])
            nc.sync.dma_start(wh_sb[:, :, 256:512], wh_r[:, :, 256:512])
            nc.scalar.dma_start(wl_sb[:, :, 256:512], wl_r[:, :, 256:512])
            nc.sync.dma_start(xh_sb[:, :, 256:512], xh_r[:, :, 256:512])
            nc.scalar.dma_start(xl_sb[:, :, 256:512], xl_r[:, :, 256:512])
            nc.sync.dma_start(mask_sb[:], mask_d[:, :, :])
            nc.scalar.dma_start(wh_sb[:, :, 512:768], wh_r[:, :, 512:768])
            nc.scalar.dma_start(wl_sb[:, :, 512:768], wl_r[:, :, 512:768])
            for c in range(1, NQC):
                cslc = slice(c * 512, (c + 1) * 512)
                nc.sync.dma_start(xh_sb[:, :, cslc], xh_r[:, :, cslc])
                nc.sync.dma_start(xl_sb[:, :, cslc], xl_r[:, :, cslc])
            nc.sync.dma_start(ident_sb[:], ident_d[:, :])
            nc.sync.dma_start(wo_sb[:], wo_d[:, :, :])

            # ---- unified pipeline ------------------------------------
            # One PSUM pool: tag "fast" (2 bufs x 2 banks) cycles the
            # short-lived tiles (QK/V projection chains, score tiles,
            # transpose outputs, out-proj accumulators); tag "acc"
            # (2 bufs x 2 banks) holds the PV^T accumulators. Projection
            # chains for chunk c+1, normalize/transpose and out-projection
            # for chunk qc-1 are interleaved into chunk qc's attention
            # kt-loop as PE filler so the exp latency on ACT is hidden.
            with (
                tc.tile_pool(name="ps", bufs=2, space="PSUM") as psp,
                tc.tile_pool(name="pt", bufs=8) as ptp,
                tc.tile_pool(name="nrm", bufs=2) as nrm,
                tc.tile_pool(name="osb", bufs=4) as osb,
                nc.allow_low_precision(reason="bf16/fp8 pipeline"),
            ):
                xsb = {"h": xh_sb, "l": xl_sb}
                wsb = {"h": wh_sb, "l": wl_sb}

                def dr_terms(lhs_of, rhs_of, ps):
                    """3-term DoubleRow accumulation into psum region ps."""
                    terms = [("h", "h"), ("l", "h"), ("h", "l")]
                    n = len(terms) * 4
                    i = 0
                    for tl, tr in terms:
                        for k2 in range(4):
                            nc.tensor.matmul(
                                ps, lhs_of(tl, k2), rhs_of(tr, k2),
                                start=(i == 0), stop=(i == n - 1),
                                perf_mode=DR)
                            i += 1

                def fast_tile(name):
                    return psp.tile([128, 2, 512], F32, name=name, tag="fast",
                                    bufs=3)

                def emit_qk_chain(c, p, lo=0, hi=512):
                    cslc = slice(c * 512 + lo, c * 512 + hi)
                    pslc = slice(p * 128, (p + 1) * 128)
                    kslc = slice(CL + p * 128, CL + (p + 1) * 128)
                    ps = fast_tile("psqk")
                    dr_terms(
                        lambda t, k2: wsb[t][:, 2 * k2:2 * k2 + 2, pslc],
                        lambda t, k2: xsb[t][:, 2 * k2:2 * k2 + 2, cslc],
                        ps[:, 0, lo:hi])
                    dr_terms(
                        lambda t, k2: wsb[t][:, 2 * k2:2 * k2 + 2, kslc],
                        lambda t, k2: xsb[t][:, 2 * k2:2 * k2 + 2, cslc],
                        ps[:, 1, lo:hi])
                    nc.vector.tensor_copy(qt_sb[p][:, cslc], ps[:, 0, lo:hi])
                    nc.vector.tensor_copy(kt_sb[p][:, cslc], ps[:, 1, lo:hi])

                def emit_v_chain(st):
                    sslc = slice(st * 128, (st + 1) * 128)
                    vslc = slice(2 * CL, 3 * CL)
                    ps = fast_tile("psv")
                    dr_terms(
                        lambda t, k2: xsb[t][:, 2 * k2:2 * k2 + 2, sslc],
                        lambda t, k2: wsb[t][:, 2 * k2:2 * k2 + 2, vslc],
                        ps[:, 0, 0:256])
                    nc.vector.tensor_copy(
                        vp_sb[:, st, :]
                        .rearrange("p (h e) -> p h e", e=65)[:, :, 0:64],
                        ps[:, 0, 0:256].rearrange("p (h d) -> p h d", d=64))

                def proj_items(c):
                    # qk chains only -- the V' chains for chunk c are emitted
                    # inside chunk c's own attention stream (they are only
                    # needed by PV^T at kt >= 4c), keeping more PE filler in
                    # the late, ACT-bound chunks.
                    return [lambda p=p: emit_qk_chain(c, p) for p in range(2)]

                def emit_st(qc, p, kt):
                    """scores + exp + mask for one key tile -> bf16 P."""
                    qlo = qc * 512
                    dq = max(0, kt * 128 - qlo)
                    s0 = dq             # bf16 scores: exact diagonal trim
                    ST = fast_tile("ST")
                    for hi in range(2):
                        hslc = slice(hi * 64, (hi + 1) * 64)
                        nc.tensor.matmul(
                            ST[:, hi, s0:],
                            kt_sb[p][hslc, kt * 128:(kt + 1) * 128],
                            qt_sb[p][hslc, qc * 512 + s0:(qc + 1) * 512],
                            start=True, stop=True)
                    PT = ptp.tile([128, 2, 512], BF16, tag="pt")
                    nc.scalar.activation(PT[:, :, dq:], ST[:, :, dq:],
                                         AF.Exp, scale=SCALE)
                    if dq > 0:      # diagonal block: mask keys > query
                        nc.gpsimd.tensor_mul(
                            PT[:, :, dq:dq + 128],
                            PT[:, :, dq:dq + 128], mask_sb[:])
                    elif kt * 128 == qlo:
                        nc.gpsimd.tensor_mul(
                            PT[:, :, 0:128],
                            PT[:, :, 0:128], mask_sb[:])
                    return PT

                def emit_pvT(qc, p, kt, PT, OTP):
                    # transposed PV: OT[q, c] += P[k, q].T @ [V|1]
                    # P tile is the 128-wide stationary; V' streams 65 cols.
                    # col 64 of each head block = softmax denominator.
                    # OTP is [128, 4, 2, 128] f32 = exactly 2 psum banks with
                    # each (j, hi) slice 512B-aligned; one accumulation group
                    # per bank: start on the bank's first matmul (lazy-zeroes
                    # the whole bank), stop on its last (diagonal of the
                    # bank's last q-tile, hi=1).
                    for j in range(4):
                        qt = 4 * qc + j
                        if qt < kt:
                            continue
                        for hi in range(2):
                            bc = (2 * p + hi) * 65
                            nc.tensor.matmul(
                                OTP[:, j, hi, 0:65],
                                PT[:, hi, j * 128:(j + 1) * 128],
                                vp_sb[:, kt, bc:bc + 65],
                                start=(kt == 0 and hi == 0 and j % 2 == 0),
                                stop=(kt == qt and hi == 1 and j % 2 == 1))

                state = {}

                def norm_items(qc, otps):
                    # O^T[q, c] = OT[q, c] * (1/denom[q]) -- denominators sit
                    # on the partition (query) axis, so this is the HW-native
                    # per-partition tensor_scalar scale; then transpose back
                    # to [c, q] on PE via the identity stationary and copy
                    # PSUM->SBUF.
                    def item_a(p, j=None):
                        # j=None: whole slab; else one q-tile piece (used
                        # for p=1 as each diagonal PV^T lands, so the qc
                        # tail only waits on the last piece)
                        if (qc, p) in state:
                            rec, Ob = state[(qc, p)]
                        else:
                            rec = nrm.tile([128, 4, 2, 1], F32, name="rec",
                                           tag="rec")
                            Ob = nrm.tile([128, 4, 2, 64], BF16, name="Ob",
                                          tag="ob")
                            state[(qc, p)] = (rec, Ob)
                        OTP = otps[p]
                        if j is None:
                            nc.vector.reciprocal(rec[:], OTP[:, :, :, 64:65])
                            js = range(4)
                        else:
                            nc.vector.reciprocal(rec[:, j, :, :],
                                                 OTP[:, j, :, 64:65])
                            js = (j,)
                        for jj in js:
                            for hi in range(2):
                                nc.vector.tensor_scalar_mul(
                                    Ob[:, jj, hi, :], OTP[:, jj, hi, 0:64],
                                    rec[:, jj, hi, :])

                    def item_b(p):
                        _, Ob = state.pop((qc, p))
                        if p == 0:
                            state[qc] = osb.tile([128, 2, 512], BF16,
                                                 name="OS2", tag="os")
                        OS2 = state[qc]
                        Tp = psp.tile([128, 4, 128], BF16, name="Tp",
                                      tag="fast", bufs=3)
                        for j in range(4):
                            nc.tensor.transpose(
                                Tp[:, j, :],
                                Ob[:, j, :, :].rearrange("p a b -> p (a b)"),
                                ident_sb[:])
                        nc.vector.tensor_copy(
                            OS2[:, p, :],
                            Tp[:].rearrange("p a b -> p (a b)"))
                    return [item_a, item_b]

                def outproj_items(qc):
                    def item(st4):
                        OS2 = state[qc]
                        sslc = slice(st4 * 128, (st4 + 1) * 128)
                        yp = fast_tile("yp")
                        ysb = osb.tile([128, 2, 512], BF16, name="ysb",
                                       tag="ys")
                        tail = qc == NQC - 1 and st4 == 3
                        for nch in range(2):
                            for kp in range(2):
                                nc.tensor.matmul(
                                    yp[:, nch, :], OS2[:, kp, sslc],
                                    wo_sb[:, kp, nch * 512:(nch + 1) * 512],
                                    start=(kp == 0), stop=(kp == 1))
                            # in the tail (last chunk) ACT is idle: split
                            # the copies across engines so the tail is not
                            # DVE-serial, and store each half as soon as
                            # it is ready on two issue engines
                            if qc == NQC - 1 and nch == 0:
                                nc.scalar.copy(ysb[:, nch, :], yp[:, nch, :])
                            else:
                                nc.vector.tensor_copy(ysb[:, nch, :],
                                                      yp[:, nch, :])
                            if tail:
                                eng = nc.scalar if nch == 0 else nc.sync
                                eng.dma_start(
                                    y_d[2 * qc + 1, 1, :, nch],
                                    ysb[:, nch, :])
                        if not tail:
                            nc.sync.dma_start(y_d[2 * qc + st4 // 2, st4 % 2],
                                              ysb[:])
                        if st4 == 3:
                            state.pop(qc)
                    return [lambda s=s: item(s) for s in range(4)]

                # ---- master loop --------------------------------------
                # preamble: chunk-0 Q/K chains. The p=0 chain is emitted as
                # quarter chains in DMA-arrival order (Q[0:256], Q[256:512],
                # K[0:256], K[256:512]) so the PE starts as soon as the
                # first 256-token x and w[Q] pieces land. Within each psum
                # bank only the first group carries start=True (its start
                # zeroes the whole bank; the second group accumulates onto
                # those zeroes) and only the last carries stop=True.
                ps00 = fast_tile("ps00")
                for (lo, hi), (row, wlo) in (
                        ((0, 256), (0, 0)), ((0, 256), (1, CL)),
                        ((256, 512), (0, 0)), ((256, 512), (1, CL))):
                        i = 0
                        for tl, tr in (("h", "h"), ("l", "h"), ("h", "l")):
                            for k2 in range(4):
                                nc.tensor.matmul(
                                    ps00[:, row, lo:hi],
                                    wsb[tl][:, 2 * k2:2 * k2 + 2,
                                            wlo:wlo + 128],
                                    xsb[tr][:, 2 * k2:2 * k2 + 2, lo:hi],
                                    start=(lo == 0 and i == 0),
                                    stop=(lo == 256 and i == 11),
                                    perf_mode=DR)
                                i += 1
                nc.vector.tensor_copy(qt_sb[0][:, 0:512], ps00[:, 0, :])
                nc.vector.tensor_copy(kt_sb[0][:, 0:512], ps00[:, 1, :])
                emit_qk_chain(0, 1)
                filler = []
                for qc in range(NQC):
                    ktmax = 4 * (qc + 1)
                    if qc + 1 < NQC:
                        filler.extend(proj_items(qc + 1))
                    # p0's accumulator is fully drained (normalized) before
                    # p1's first PV^T in the unified stream, so one "acc"
                    # slot suffices; the freed banks give "fast" a 3rd slot.
                    otps = [psp.tile([128, 4, 2, 128], F32, name=f"OT{qc}{p}",
                                     tag="acc", bufs=1) for p in range(2)]
                    # one (p, kt) stream per qc: scores/exp run 2 key tiles
                    # ahead of PV^T across the p boundary, so the pipeline
                    # drains only once per qc; filler lands between ST and
                    # PV so PE has work while ACT computes exp. p=0's
                    # normalize/transpose is appended as soon as its last
                    # PV^T is emitted, trimming the qc tail.
                    seq = [(p, kt) for p in range(2) for kt in range(ktmax)]
                    n_iters = len(seq) + 1
                    n_fill = len(filler)
                    fi = 0
                    pts = {}
                    nrm_it = norm_items(qc, otps)
                    # this chunk's V' chains, placed early in the stream
                    # (PV^T first touches V'[4qc+j] at index 4qc+j+2)
                    vch = {(j if qc == 0 else 2 * j):
                           (lambda st=4 * qc + j: emit_v_chain(st))
                           for j in range(4)}
                    LAG = 4 if qc else 3
                    for i in range(len(seq) + LAG):
                        if i < len(seq):
                            p, kt = seq[i]
                            pts[(p, kt)] = emit_st(qc, p, kt)
                        if i in vch:
                            vch.pop(i)()
                        while fi < n_fill and \
                                fi * n_iters < (i + 1) * n_fill:
                            filler[fi]()
                            fi += 1
                        if i >= LAG:
                            pp_, kk_ = seq[i - LAG]
                            emit_pvT(qc, pp_, kk_, pts.pop((pp_, kk_)),
                                     otps[pp_])
                            item_a, item_b = nrm_it
                            if (pp_, kk_) == (0, ktmax - 1):
                                item_a(0)       # normalize p=0 (DVE)
                            elif (pp_, kk_) == (1, 1):
                                item_b(0)       # transpose p=0 (PE)
                    while fi < n_fill:
                        filler[fi]()
                        fi += 1
                    filler = [lambda f=nrm_it[0]: f(1),
                              lambda f=nrm_it[1]: f(1)] + outproj_items(qc)
                # tail: last chunk's normalize + out-projection
                for it in filler:
                    it()

    if split_waits:
        _split_excess_waits(nc, max_waits=1)
    return nc


_NC = None


def _fp8_split(a):
    hi = a.astype(ml_dtypes.float8_e4m3)
    lo = (a - hi.astype(np.float32)).astype(ml_dtypes.float8_e4m3)
    return hi, lo


def _core_in_map(inputs, core, _xs_cache={}):
    x = np.asarray(inputs["x"], dtype=np.float32)
    Wq, Wk, Wv, Wo = (np.asarray(inputs[k], dtype=np.float32)
                      for k in ("Wq", "Wk", "Wv", "Wo"))
    b, g = divmod(core, G)
    csl = slice(g * CL, (g + 1) * CL)
    key = id(inputs)
    if key not in _xs_cache:
        _xs_cache.clear()
        _xs_cache[key] = [_fp8_split(np.ascontiguousarray(x[bb].T))
                          for bb in range(B)]
    xs = _xs_cache[key]
    w = np.concatenate(
        [Wq[csl, :].T, Wk[csl, :].T, Wv[csl, :].T], axis=1) * WSCALE
    whi, wlo = _fp8_split(np.ascontiguousarray(w))
    wo = np.ascontiguousarray(
        Wo[:, csl].T.reshape(2, 128, D).transpose(1, 0, 2)) / WSCALE
    tri = np.triu(np.ones((128, 128), dtype=np.float32))  # m[k,q] = k<=q
    mask16 = np.ascontiguousarray(
        np.stack([tri, tri], axis=1)).astype(ml_dtypes.bfloat16)
    ident = np.eye(128, dtype=np.float32).astype(ml_dtypes.bfloat16)
    return {
        "xh": xs[b][0], "xl": xs[b][1],
        "wh": whi, "wl": wlo,
        "wo": wo.astype(ml_dtypes.bfloat16),
        "mask": mask16,
        "ident": ident,
    }


def kernel(x, Wq, Wk, Wv, Wo):
    global _NC
    if _NC is None:
        _NC = _build_nc()
    inputs = {"x": x, "Wq": Wq, "Wk": Wk, "Wv": Wv, "Wo": Wo}
    in_maps = [_core_in_map(inputs, core) for core in range(8)]
    res = run_bass_kernel_spmd(_NC, in_maps, list(range(8)))
    y = np.empty((B, S, D), dtype=np.float32)
    for b in range(B):
        acc = np.zeros((S // 256, 2, 128, 2, 512), dtype=np.float32)
        for g in range(G):
            acc += res.results[4 * b + g]["y"].astype(np.float32)
        y[b] = acc.reshape(S, D)
    return y
